# revision 1
# baseline (speedup 1.0000x reference)
# GCN 3-layer + global max pool on 8 NeuronCores (Trainium2, Bass/Tile).
#
# Sharding: nodes are assigned to 8 cores (degree-balanced), edges partitioned
# by destination node. Each layer: every core holds the full transformed+
# dinv-scaled feature table g in HBM, gathers g[src] for its own edges with
# dma_gather (512B descriptors), aggregates per dst-tile with one-hot matmuls
# on the PE (feature-major), applies bias, transforms with the next weight
# matrix (node-major), scales by dinv, and AllGathers the new table.
# Final layer: Wlin^T @ h3^T, dummy slots masked with a rank-1 matmul,
# reduce_max over nodes -> per-core [64,1] partials, max over cores on host.

import numpy as np

N, E, F, H, O = 50000, 800000, 256, 128, 64
NCORES = 8
TILE_D = 256                      # dst nodes per aggregation tile
TILES_PER_CORE = 25
NPC = TILE_D * TILES_PER_CORE     # 6400 node slots per core
NPAD = NPC * NCORES               # 51200
HALF = NPAD // 2                  # 25600 (< 32768 so int16 gather idx works)
CHUNK = 128
NEG_BIG = -1.0e30

_compiled_cache = {}


# ---------------------------------------------------------------- host prep

def _preprocess(x, edge_index, W0, b0, W1, b1, W2, b2, Wlin, blin):
    """Returns (meta, in_maps). meta drives the static program structure and
    must be identical across cores; in_maps hold per-core data."""
    n = x.shape[0]
    x = np.asarray(x, np.float32)
    ei = np.asarray(edge_index, np.int64)

    # self-loops + degrees (matches reference _gcn_norm)
    loop = np.arange(n, dtype=np.int64)
    src = np.concatenate([ei[0], loop])
    dst = np.concatenate([ei[1], loop])
    deg = np.bincount(dst, minlength=n).astype(np.float32)
    dinv = np.where(deg > 0, 1.0 / np.sqrt(deg), 0.0).astype(np.float32)

    # node -> slot permutation: snake over (core,tile) buckets by degree
    nbuckets = NCORES * TILES_PER_CORE
    order = np.argsort(-deg, kind="stable")
    slot_of = np.full(n, -1, np.int64)
    fill = np.zeros(nbuckets, np.int64)
    for i, v in enumerate(order):
        p, q = divmod(i, nbuckets)
        b = q if (p % 2 == 0) else nbuckets - 1 - q
        s = b * TILE_D + fill[b]
        assert fill[b] < TILE_D, "bucket overflow"
        fill[b] += 1
        slot_of[v] = s
    node_at = np.full(NPAD, -1, np.int64)
    node_at[slot_of] = np.arange(n)

    # edge fields in slot space
    s_slot = slot_of[src]
    d_slot = slot_of[dst]
    e_core = d_slot // NPC
    e_tile = (d_slot % NPC) // TILE_D
    e_half = (s_slot >= HALF).astype(np.int64)
    e_gidx = (s_slot % HALF).astype(np.int64)
    e_dloc = (d_slot % TILE_D).astype(np.float32)
    e_ddv = dinv[dst]

    gid = (e_core * TILES_PER_CORE + e_tile) * 2 + e_half
    ordr = np.argsort(gid, kind="stable")
    gid_s = gid[ordr]
    cnt = np.bincount(gid_s, minlength=NCORES * TILES_PER_CORE * 2)
    cnt = cnt.reshape(NCORES, TILES_PER_CORE, 2)
    starts = np.zeros(NCORES * TILES_PER_CORE * 2 + 1, np.int64)
    np.cumsum(cnt.reshape(-1), out=starts[1:])

    # static chunk counts per (tile, half): max over cores
    n_ch = np.maximum(1, (cnt.max(axis=0) + CHUNK - 1) // CHUNK)  # [T,2]
    n_lo = n_ch[:, 0].astype(int).tolist()
    n_hi = n_ch[:, 1].astype(int).tolist()
    totch = int(sum(n_lo) + sum(n_hi))

    gidx_s = e_gidx[ordr]
    dloc_s = e_dloc[ordr]
    ddv_s = e_ddv[ordr]

    in_maps = []
    for c in range(NCORES):
        idx_arr = np.zeros((16, totch * 8), np.int16)
        dloc_arr = np.full((CHUNK, totch), -1.0, np.float32)
        ddv_arr = np.zeros((CHUNK, totch), np.float32)
        cbase = 0
        for t in range(TILES_PER_CORE):
            for hf in range(2):
                g = (c * TILES_PER_CORE + t) * 2 + hf
                a, bnd = starts[g], starts[g + 1]
                m = int(bnd - a)
                nch = n_lo[t] if hf == 0 else n_hi[t]
                nslot = nch * CHUNK
                gi = np.zeros(nslot, np.int64)
                dl = np.full(nslot, -1.0, np.float32)
                dv = np.zeros(nslot, np.float32)
                gi[:m] = gidx_s[a:bnd]
                dl[:m] = dloc_s[a:bnd]
                dv[:m] = ddv_s[a:bnd]
                # idx wrap: logical i -> [i%16, off + i//16]
                idx_arr[:, cbase * 8: cbase * 8 + nslot // 16] = (
                    gi.reshape(-1, 16).T.astype(np.int16))
                dloc_arr[:, cbase: cbase + nch] = dl.reshape(nch, CHUNK).T
                ddv_arr[:, cbase: cbase + nch] = dv.reshape(nch, CHUNK).T
                cbase += nch
        idx_full = np.tile(idx_arr, (8, 1))  # replicate 16-row pattern x8

        nodes_c = node_at[c * NPC:(c + 1) * NPC]
        valid = nodes_c >= 0
        xt = np.zeros((F, NPC), np.float32)
        xt[:, valid] = (x[nodes_c[valid]] * dinv[nodes_c[valid], None]).T
        dvc = np.zeros(NPC, np.float32)
        dvc[valid] = dinv[nodes_c[valid]]
        dinvcols = dvc.reshape(NPC // CHUNK, CHUNK).T.copy()
        dmask = (~valid).astype(np.float32).reshape(1, NPC)

        iota = np.tile(np.arange(TILE_D, dtype=np.float32), (CHUNK, 1))
        in_maps.append({
            "xt": xt,
            "w0": np.asarray(W0, np.float32),
            "w1": np.asarray(W1, np.float32),
            "w2": np.asarray(W2, np.float32),
            "wl": np.asarray(Wlin, np.float32),
            "b0": np.asarray(b0, np.float32).reshape(H, 1),
            "b1": np.asarray(b1, np.float32).reshape(H, 1),
            "b2": np.asarray(b2, np.float32).reshape(H, 1),
            "iota": iota,
            "negbig": np.full((1, O), NEG_BIG, np.float32),
            "idx": idx_full,
            "dstloc": dloc_arr,
            "dinvdst": ddv_arr,
            "dinvcols": dinvcols,
            "dmask": dmask,
        })

    has_bias01 = bool(np.any(np.asarray(b0) != 0) or np.any(np.asarray(b1) != 0))
    if not has_bias01:
        for m in in_maps:
            m["dinvcols"] = m["dinvcols"] ** 2
    meta = (tuple(n_lo), tuple(n_hi), has_bias01)
    return meta, in_maps


# ---------------------------------------------------------------- device

def _build(meta, reps=1):
    import os
    import concourse.bacc as bacc
    import concourse.tile as tile
    from concourse import mybir

    dbg_nogather = os.environ.get("K_NOGATHER") == "1"
    n_queues = int(os.environ.get("K_NQUEUES", "4"))
    dbg_nocoll = os.environ.get("K_NOCOLL") == "1"
    dbg_nomask = os.environ.get("K_NOMASK") == "1"
    dbg_onlygather = os.environ.get("K_ONLYGATHER") == "1"
    dbg_noagg = os.environ.get("K_NOAGG") == "1"
    dbg_nop = os.environ.get("K_NOP") == "1"
    use_bf16 = os.environ.get("K_BF16", "1") == "1"

    n_lo, n_hi, has_bias01 = meta
    T = TILES_PER_CORE
    totch = sum(n_lo) + sum(n_hi)
    f32 = mybir.dt.float32
    f32r = mybir.dt.float32r
    bf16 = mybir.dt.bfloat16
    i16 = mybir.dt.int16
    dt_g = bf16 if use_bf16 else f32r
    KH = F // H  # k-halves for the input transform

    nc = bacc.Bacc("TRN2", target_bir_lowering=False, debug=False,
                   enable_asserts=False, num_devices=NCORES,
                   num_swdge_queues=n_queues,
                   dynamic_dma_scratch_size=int(
                       os.environ.get("K_SCRATCH", "49152")))

    xt_d = nc.dram_tensor("xt", [F, NPC], f32, kind="ExternalInput")
    w0_d = nc.dram_tensor("w0", [F, H], f32, kind="ExternalInput")
    w1_d = nc.dram_tensor("w1", [H, H], f32, kind="ExternalInput")
    w2_d = nc.dram_tensor("w2", [H, H], f32, kind="ExternalInput")
    wl_d = nc.dram_tensor("wl", [H, O], f32, kind="ExternalInput")
    b_d = [nc.dram_tensor(f"b{i}", [H, 1], f32, kind="ExternalInput")
           for i in range(3)]
    iota_d = nc.dram_tensor("iota", [CHUNK, TILE_D], f32, kind="ExternalInput")
    neg_d = nc.dram_tensor("negbig", [1, O], f32, kind="ExternalInput")
    idx_d = nc.dram_tensor("idx", [CHUNK, totch * 8], i16, kind="ExternalInput")
    dloc_d = nc.dram_tensor("dstloc", [CHUNK, totch], f32, kind="ExternalInput")
    ddv_d = nc.dram_tensor("dinvdst", [CHUNK, totch], f32, kind="ExternalInput")
    dvc_d = nc.dram_tensor("dinvcols", [CHUNK, NPC // CHUNK], f32,
                           kind="ExternalInput")
    dmask_d = nc.dram_tensor("dmask", [1, NPC], f32, kind="ExternalInput")
    out_d = nc.dram_tensor("out", [O, 1], f32, kind="ExternalOutput")

    dt_mem = bf16 if use_bf16 else f32
    gsl = [nc.dram_tensor(f"gsl{i}", [NPC, H], dt_mem, kind="Internal")
           for i in range(3)]
    gfull = [nc.dram_tensor(f"gfull{i}", [NPAD, H], dt_mem, kind="Internal",
                            addr_space="Shared") for i in range(3)]

    with tile.TileContext(nc) as tc:
        from concourse.library_config import mlp as _mlp_lib
        nc.gpsimd.load_library(_mlp_lib)
        with (
            tc.tile_pool(name="const", bufs=1) as cpool,
            tc.tile_pool(name="psagg", bufs=2, space="PSUM") as psagg,
            tc.tile_pool(name="pstr", bufs=2, space="PSUM") as pstr,
            tc.tile_pool(name="psfin", bufs=2, space="PSUM") as psfin,
        ):
            # resident constants
            w0_sb = [cpool.tile([CHUNK, H], f32, name=f"w0_{k}")
                     for k in range(KH)]
            for k in range(KH):
                nc.sync.dma_start(w0_sb[k][:, :],
                                  w0_d.ap()[k * CHUNK:(k + 1) * CHUNK, :])
            w1_sb = cpool.tile([H, H], f32, name="w1")
            nc.sync.dma_start(w1_sb[:, :], w1_d.ap()[:, :])
            w2_sb = cpool.tile([H, H], f32, name="w2")
            nc.sync.dma_start(w2_sb[:, :], w2_d.ap()[:, :])
            wl_sb = cpool.tile([H, O], f32, name="wl")
            nc.sync.dma_start(wl_sb[:, :], wl_d.ap()[:, :])
            b_sb = []
            for i in range(3):
                bt = cpool.tile([H, 1], f32, name=f"b{i}")
                nc.sync.dma_start(bt[:, :], b_d[i].ap()[:, :])
                b_sb.append(bt)
            iota_sb = cpool.tile([CHUNK, TILE_D], f32, name="iota")
            nc.sync.dma_start(iota_sb[:, :], iota_d.ap()[:, :])
            neg_sb = cpool.tile([1, O], f32, name="negbig")
            nc.sync.dma_start(neg_sb[:, :], neg_d.ap()[:, :])
            idx_sb = cpool.tile([CHUNK, totch * 8], i16, name="idx")
            nc.sync.dma_start(idx_sb[:, :], idx_d.ap()[:, :])
            dloc_sb = cpool.tile([CHUNK, totch], f32, name="dstloc")
            nc.sync.dma_start(dloc_sb[:, :], dloc_d.ap()[:, :])
            ddv_sb = cpool.tile([CHUNK, totch], f32, name="dinvdst")
            nc.sync.dma_start(ddv_sb[:, :], ddv_d.ap()[:, :])
            dvc_sb = cpool.tile([CHUNK, NPC // CHUNK], f32, name="dinvcols")
            nc.sync.dma_start(dvc_sb[:, :], dvc_d.ap()[:, :])
            dmask_sb = cpool.tile([1, NPC], f32, name="dmask")
            nc.sync.dma_start(dmask_sb[:, :], dmask_d.ap()[:, :])
            runmax = cpool.tile([O, T], f32, name="runmax")
            constP = None
            if dbg_nop:
                constP = cpool.tile([CHUNK, TILE_D], dt_g, name="constP")
                nc.vector.tensor_scalar(
                    constP[:, :], iota_sb[:, :], dloc_sb[:, 0:1],
                    ddv_sb[:, 0:1], mybir.AluOpType.is_equal,
                    mybir.AluOpType.mult)
            if dbg_onlygather:
                nc.vector.memset(runmax[:, :], 0.0)

            # ---- init transform (once): g0 = (dinv*x) @ W0, node-major.
            # xt lives in its own pool so the space is recycled for the
            # per-layer pools below.
            with (tc.tile_pool(name="xtp", bufs=1) as xtpool,
                  tc.tile_pool(name="goutp", bufs=4) as opool0):
                xt_sb = [xtpool.tile([CHUNK, NPC], f32, name=f"xt{k}")
                         for k in range(KH)]
                for k in range(KH):
                    nc.sync.dma_start(xt_sb[k][:, :],
                                      xt_d.ap()[k * CHUNK:(k + 1) * CHUNK, :])
                for j in range(NPC // CHUNK):
                    ps = pstr.tile([CHUNK, H], f32)
                    for k in range(KH):
                        nc.tensor.matmul(
                            ps[:, :],
                            lhsT=xt_sb[k][:, j * CHUNK:(j + 1) * CHUNK],
                            rhs=w0_sb[k][:, :],
                            start=(k == 0), stop=(k == KH - 1))
                    gc = opool0.tile([CHUNK, H], dt_mem)
                    nc.scalar.activation(gc[:, :], ps[:, :],
                                         mybir.ActivationFunctionType.Copy)
                    nc.sync.dma_start(
                        gsl[0].ap()[j * CHUNK:(j + 1) * CHUNK, :], gc[:, :])
            if dbg_nocoll:
                for cc in range(NCORES):
                    nc.sync.dma_start(
                        gfull[0].ap()[cc * NPC:(cc + 1) * NPC, :],
                        gsl[0].ap()[:, :])
            else:
                nc.gpsimd.collective_compute(
                    "AllGather", mybir.AluOpType.bypass,
                    replica_groups=[list(range(NCORES))],
                    ins=[gsl[0].ap()], outs=[gfull[0].ap()])

            with (
                tc.tile_pool(name="gather", bufs=(5 if use_bf16 else 4)) as gpool,
                tc.tile_pool(name="pmat", bufs=3) as ppool,
                tc.tile_pool(name="stile", bufs=3) as spool,
                tc.tile_pool(name="gout", bufs=4) as opool,
            ):
                W_next = {0: w1_sb, 1: w2_sb}
                for _rep in range(reps):
                    for l in range(3):
                        cbase = 0
                        idxoff = 0
                        gq = 0
                        for t in range(T):
                            nch_t = [n_lo[t], n_hi[t]]
                            gt = []
                            pms = []
                            cb_hf = [cbase, cbase + nch_t[0]]
                            for hf in range(2):
                                nch = nch_t[hf]
                                gtile = gpool.tile([CHUNK, nch * H], dt_g,
                                                   tag=f"g{hf}")
                                nid = nch * CHUNK
                                if dbg_nogather:
                                    nc.sync.dma_start(
                                        gtile[:, :].rearrange(
                                            "p (c e) -> p c e", e=H),
                                        gfull[l].ap()[:nid, :].bitcast(dt_g)
                                        .rearrange("(c p) e -> p c e", p=CHUNK))
                                else:
                                    nc.gpsimd.dma_gather(
                                        out_ap=gtile[:, :].rearrange(
                                            "p (c e) -> p c e", e=H),
                                        in_ap=gfull[l].ap()
                                        [hf * HALF:(hf + 1) * HALF, :]
                                        .bitcast(dt_g),
                                        idxs_ap=idx_sb[:, idxoff:
                                                       idxoff + nid // 16],
                                        num_idxs=nid, num_idxs_reg=nid,
                                        elem_size=H, single_packet=False,
                                        queue_num=gq % n_queues)
                                gq += 1
                                gt.append(gtile)
                                idxoff += nid // 16
                                use_multi = (l == 2) or not has_bias01
                                if use_multi and not (dbg_onlygather or
                                                      dbg_noagg or dbg_nop):
                                    Pm = ppool.tile(
                                        [CHUNK, nch * TILE_D], dt_g, tag="pm")
                                    pv = Pm[:, :].rearrange(
                                        "p (g d) -> p g d", d=TILE_D)
                                    dl = dloc_sb[:, cb_hf[hf]:cb_hf[hf] + nch]\
                                        .broadcast_to([CHUNK, nch, TILE_D])
                                    ioap = iota_sb[:, :]
                                    io = ioap.__replace__(
                                        ap=[list(ioap.ap[0]), [0, nch],
                                            list(ioap.ap[1])])
                                    nc.vector.scalar_tensor_tensor(
                                        pv, dl, 0.0, io,
                                        mybir.AluOpType.add,
                                        mybir.AluOpType.is_equal)
                                    if l == 2:
                                        dv = ddv_sb[:, cb_hf[hf]:
                                                    cb_hf[hf] + nch]\
                                            .broadcast_to(
                                                [CHUNK, nch, TILE_D])
                                        nc.vector.tensor_tensor(
                                            out=pv, in0=pv, in1=dv,
                                            op=mybir.AluOpType.mult)
                                    pms.append(Pm)
                                else:
                                    pms.append(None)
                            ps = psagg.tile([H, TILE_D], f32)
                            ntot = nch_t[0] + nch_t[1]
                            if dbg_onlygather:
                                cbase += ntot
                                continue
                            if dbg_noagg:
                                # skip P-builds and chunk matmuls; fake psum
                                nc.tensor.matmul(
                                    ps[:, :], lhsT=gt[0][:, 0:H],
                                    rhs=gt[0][:, 0:TILE_D],
                                    start=True, stop=True)
                            for k in range([0, ntot][not dbg_noagg]):
                                hf = 0 if k < nch_t[0] else 1
                                kk = k if hf == 0 else k - nch_t[0]
                                cg = cbase + k
                                if dbg_nop:
                                    P = constP
                                elif pms[hf] is not None:
                                    P = pms[hf][:, kk * TILE_D:
                                                (kk + 1) * TILE_D]
                                else:
                                    P = ppool.tile([CHUNK, TILE_D], dt_g)
                                    nc.vector.tensor_scalar(
                                        P[:, :], iota_sb[:, :],
                                        dloc_sb[:, cg:cg + 1],
                                        ddv_sb[:, cg:cg + 1],
                                        mybir.AluOpType.is_equal,
                                        mybir.AluOpType.mult)
                                nc.tensor.matmul(
                                    ps[:, :],
                                    lhsT=gt[hf][:, kk * H:(kk + 1) * H],
                                    rhs=P if pms[hf] is not None and not dbg_nop
                                    else P[:, :],
                                    start=(k == 0), stop=(k == ntot - 1))
                            cbase += ntot
                            # evict: h^T = psum + b_l (bias per feat partition)
                            sT = spool.tile([H, TILE_D], f32)
                            if l == 2 or has_bias01:
                                nc.scalar.activation(
                                    sT[:, :], ps[:, :],
                                    mybir.ActivationFunctionType.Identity,
                                    bias=b_sb[l][:, 0:1])
                            else:
                                nc.scalar.activation(
                                    sT[:, :], ps[:, :],
                                    mybir.ActivationFunctionType.Copy)
                            if l < 2:
                                for j2 in range(TILE_D // CHUNK):
                                    jn = t * (TILE_D // CHUNK) + j2
                                    ps2 = pstr.tile([CHUNK, H], f32)
                                    nc.tensor.matmul(
                                        ps2[:, :],
                                        lhsT=sT[:, j2 * CHUNK:(j2 + 1) * CHUNK],
                                        rhs=W_next[l][:, :],
                                        start=True, stop=True)
                                    gc = opool.tile([CHUNK, H], dt_mem)
                                    nc.scalar.activation(
                                        gc[:, :], ps2[:, :],
                                        mybir.ActivationFunctionType.Copy,
                                        scale=dvc_sb[:, jn:jn + 1])
                                    nc.sync.dma_start(
                                        gsl[l + 1].ap()
                                        [jn * CHUNK:(jn + 1) * CHUNK, :],
                                        gc[:, :])
                            else:
                                psf = psfin.tile([O, TILE_D], f32)
                                nc.tensor.matmul(psf[:, :], lhsT=wl_sb[:, :],
                                                 rhs=sT[:, :], start=True,
                                                 stop=dbg_nomask)
                                if not dbg_nomask:
                                    nc.tensor.matmul(
                                        psf[:, :], lhsT=neg_sb[:, :],
                                        rhs=dmask_sb[0:1,
                                                     t * TILE_D:(t + 1) * TILE_D],
                                        start=False, stop=True)
                                nc.vector.reduce_max(
                                    out=runmax[:, t:t + 1], in_=psf[:, :],
                                    axis=mybir.AxisListType.X)
                        if l < 2:
                            if dbg_onlygather:
                                continue
                            if dbg_nocoll:
                                for cc in range(NCORES):
                                    nc.sync.dma_start(
                                        gfull[l + 1].ap()
                                        [cc * NPC:(cc + 1) * NPC, :],
                                        gsl[l + 1].ap()[:, :])
                            else:
                                nc.gpsimd.collective_compute(
                                    "AllGather", mybir.AluOpType.bypass,
                                    replica_groups=[list(range(NCORES))],
                                    ins=[gsl[l + 1].ap()],
                                    outs=[gfull[l + 1].ap()])

                    fin = cpool.tile([O, 1], f32, name=f"fin{_rep}")
                    nc.vector.reduce_max(out=fin[:, :], in_=runmax[:, :],
                                         axis=mybir.AxisListType.X)
                    nc.sync.dma_start(out_d.ap()[:, :], fin[:, :])

    nc.compile()
    return nc


# ---------------------------------------------------------------- entry

def kernel(x, edge_index, batch, W0, b0, W1, b1, W2, b2, Wlin, blin):
    from concourse.bass_utils import run_bass_kernel_spmd

    meta, in_maps = _preprocess(np.asarray(x), np.asarray(edge_index),
                                W0, b0, W1, b1, W2, b2, Wlin, blin)
    nc = _compiled_cache.get(meta)
    if nc is None:
        nc = _build(meta)
        _compiled_cache[meta] = nc
    res = run_bass_kernel_spmd(nc, in_maps, core_ids=list(range(NCORES)))
    parts = np.stack([r["out"][:, 0] for r in res.results])  # [cores, O]
    out = parts.max(axis=0) + np.asarray(blin, np.float32)
    return out.reshape(1, O).astype(np.float32)



# revision 12
# speedup vs baseline: 2.3574x; 2.3574x over previous
# GCN 3-layer + global max pool on 8 NeuronCores (Trainium2, Bass/Tile).
#
# Sharding: nodes are assigned to 8 cores (degree-balanced), edges partitioned
# by destination node. Each layer: every core holds the full transformed+
# dinv-scaled feature table g in HBM, gathers g[src] for its own edges with
# dma_gather (512B descriptors), aggregates per dst-tile with one-hot matmuls
# on the PE (feature-major), applies bias, transforms with the next weight
# matrix (node-major), scales by dinv, and AllGathers the new table.
# Final layer: Wlin^T @ h3^T, dummy slots masked with a rank-1 matmul,
# reduce_max over nodes -> per-core [64,1] partials, max over cores on host.

import numpy as np

N, E, F, H, O = 50000, 800000, 256, 128, 64
NCORES = 8
TILE_D = 128                      # dst nodes per aggregation tile
TILES_PER_CORE = 50
NPC = TILE_D * TILES_PER_CORE     # 6400 node slots per core
NPAD = NPC * NCORES               # 51200
HALF = NPAD // 2                  # 25600 (< 32768 so int16 gather idx works)
CHUNK = 128
NEG_BIG = -1.0e30

_compiled_cache = {}


# ---------------------------------------------------------------- host prep

def _preprocess(x, edge_index, W0, b0, W1, b1, W2, b2, Wlin, blin):
    """Returns (meta, in_maps). meta drives the static program structure and
    must be identical across cores; in_maps hold per-core data."""
    n = x.shape[0]
    x = np.asarray(x, np.float32)
    ei = np.asarray(edge_index, np.int64)

    # self-loops + degrees (matches reference _gcn_norm)
    loop = np.arange(n, dtype=np.int64)
    src = np.concatenate([ei[0], loop])
    dst = np.concatenate([ei[1], loop])
    deg = np.bincount(dst, minlength=n).astype(np.float32)
    dinv = np.where(deg > 0, 1.0 / np.sqrt(deg), 0.0).astype(np.float32)

    # node -> slot permutation: snake over (core,tile) buckets by degree
    nbuckets = NCORES * TILES_PER_CORE
    order = np.argsort(-deg, kind="stable")
    slot_of = np.full(n, -1, np.int64)
    fill = np.zeros(nbuckets, np.int64)
    for i, v in enumerate(order):
        p, q = divmod(i, nbuckets)
        b = q if (p % 2 == 0) else nbuckets - 1 - q
        s = b * TILE_D + fill[b]
        assert fill[b] < TILE_D, "bucket overflow"
        fill[b] += 1
        slot_of[v] = s
    node_at = np.full(NPAD, -1, np.int64)
    node_at[slot_of] = np.arange(n)

    # edge fields in slot space
    s_slot = slot_of[src]
    d_slot = slot_of[dst]
    e_core = d_slot // NPC
    e_tile = (d_slot % NPC) // TILE_D
    e_half = (s_slot >= HALF).astype(np.int64)
    e_gidx = (s_slot % HALF).astype(np.int64)
    e_dloc = (d_slot % TILE_D).astype(np.float32)
    e_ddv = dinv[dst]

    gid = (e_core * TILES_PER_CORE + e_tile) * 2 + e_half
    ordr = np.argsort(gid, kind="stable")
    gid_s = gid[ordr]
    cnt = np.bincount(gid_s, minlength=NCORES * TILES_PER_CORE * 2)
    cnt = cnt.reshape(NCORES, TILES_PER_CORE, 2)
    starts = np.zeros(NCORES * TILES_PER_CORE * 2 + 1, np.int64)
    np.cumsum(cnt.reshape(-1), out=starts[1:])

    # static chunk counts per (tile, half): max over cores
    n_ch = np.maximum(1, (cnt.max(axis=0) + CHUNK - 1) // CHUNK)  # [T,2]
    n_lo = n_ch[:, 0].astype(int).tolist()
    n_hi = n_ch[:, 1].astype(int).tolist()
    totch = int(sum(n_lo) + sum(n_hi))

    gidx_s = e_gidx[ordr]
    dloc_s = e_dloc[ordr]
    ddv_s = e_ddv[ordr]

    in_maps = []
    for c in range(NCORES):
        idx_arr = np.zeros((16, totch * 8), np.int16)
        dloc_arr = np.full((CHUNK, totch), -1.0, np.float32)
        ddv_arr = np.zeros((CHUNK, totch), np.float32)
        cbase = 0
        for t in range(TILES_PER_CORE):
            for hf in range(2):
                g = (c * TILES_PER_CORE + t) * 2 + hf
                a, bnd = starts[g], starts[g + 1]
                m = int(bnd - a)
                nch = n_lo[t] if hf == 0 else n_hi[t]
                nslot = nch * CHUNK
                gi = np.zeros(nslot, np.int64)
                dl = np.full(nslot, -1.0, np.float32)
                dv = np.zeros(nslot, np.float32)
                gi[:m] = gidx_s[a:bnd]
                dl[:m] = dloc_s[a:bnd]
                dv[:m] = ddv_s[a:bnd]
                # idx wrap: logical i -> [i%16, off + i//16]
                idx_arr[:, cbase * 8: cbase * 8 + nslot // 16] = (
                    gi.reshape(-1, 16).T.astype(np.int16))
                dloc_arr[:, cbase: cbase + nch] = dl.reshape(nch, CHUNK).T
                ddv_arr[:, cbase: cbase + nch] = dv.reshape(nch, CHUNK).T
                cbase += nch
        idx_full = np.tile(idx_arr, (8, 1))  # replicate 16-row pattern x8

        nodes_c = node_at[c * NPC:(c + 1) * NPC]
        valid = nodes_c >= 0
        xt = np.zeros((F, NPC), np.float32)
        xt[:, valid] = (x[nodes_c[valid]] * dinv[nodes_c[valid], None]).T
        dvc = np.zeros(NPC, np.float32)
        dvc[valid] = dinv[nodes_c[valid]]
        dinvcols = dvc.reshape(NPC // CHUNK, CHUNK).T.copy()
        dmask = (~valid).astype(np.float32).reshape(1, NPC)
        # per-dst-column dinv for the final layer's psf scale; dummies get
        # 1.0 so the NEG_BIG mask (added before the scale) survives.
        dvrow = np.where(valid, dvc, 1.0).astype(np.float32)
        dvrep = np.tile(dvrow.reshape(1, NPC), (O, 1))

        iota = np.tile(np.arange(TILE_D, dtype=np.float32), (CHUNK, 1))
        in_maps.append({
            "xt": xt,
            "w0": np.asarray(W0, np.float32),
            "w1": np.asarray(W1, np.float32),
            "w2": np.asarray(W2, np.float32),
            "wl": np.asarray(Wlin, np.float32),
            "b0": np.asarray(b0, np.float32).reshape(H, 1),
            "b1": np.asarray(b1, np.float32).reshape(H, 1),
            "b2": np.asarray(b2, np.float32).reshape(H, 1),
            "iota": iota,
            "negbig": np.full((1, O), NEG_BIG, np.float32),
            "idx": idx_full,
            "dstloc": dloc_arr,
            "dinvdst": ddv_arr,
            "dinvcols": dinvcols,
            "dmask": dmask,
            "dvrep": dvrep,
        })

    has_bias01 = bool(np.any(np.asarray(b0) != 0) or np.any(np.asarray(b1) != 0))
    b2zero = bool(np.all(np.asarray(b2) == 0))
    if not has_bias01:
        for m in in_maps:
            m["dinvcols"] = m["dinvcols"] ** 2
    meta = (tuple(n_lo), tuple(n_hi), has_bias01, b2zero)
    return meta, in_maps


# ---------------------------------------------------------------- device

def _build(meta, reps=1):
    import os
    import concourse.bacc as bacc
    import concourse.tile as tile
    from concourse import mybir

    dbg_nogather = os.environ.get("K_NOGATHER") == "1"
    n_queues = int(os.environ.get("K_NQUEUES", "4"))
    dbg_nocoll = os.environ.get("K_NOCOLL") == "1"
    dbg_nomask = os.environ.get("K_NOMASK") == "1"
    dbg_onlygather = os.environ.get("K_ONLYGATHER") == "1"
    dbg_noagg = os.environ.get("K_NOAGG") == "1"
    dbg_nop = os.environ.get("K_NOP") == "1"
    use_bf16 = os.environ.get("K_BF16", "1") == "1"

    n_lo, n_hi, has_bias01, b2zero = meta
    T = TILES_PER_CORE
    totch = sum(n_lo) + sum(n_hi)
    f32 = mybir.dt.float32
    f32r = mybir.dt.float32r
    bf16 = mybir.dt.bfloat16
    i16 = mybir.dt.int16
    dt_g = bf16 if use_bf16 else f32r
    KH = F // H  # k-halves for the input transform

    nc = bacc.Bacc("TRN2", target_bir_lowering=False, debug=False,
                   enable_asserts=False, num_devices=NCORES,
                   num_swdge_queues=n_queues,
                   dynamic_dma_scratch_size=int(
                       os.environ.get("K_SCRATCH", "49152")))

    xt_d = nc.dram_tensor("xt", [F, NPC], f32, kind="ExternalInput")
    w0_d = nc.dram_tensor("w0", [F, H], f32, kind="ExternalInput")
    w1_d = nc.dram_tensor("w1", [H, H], f32, kind="ExternalInput")
    w2_d = nc.dram_tensor("w2", [H, H], f32, kind="ExternalInput")
    wl_d = nc.dram_tensor("wl", [H, O], f32, kind="ExternalInput")
    b_d = [nc.dram_tensor(f"b{i}", [H, 1], f32, kind="ExternalInput")
           for i in range(3)]
    iota_d = nc.dram_tensor("iota", [CHUNK, TILE_D], f32, kind="ExternalInput")
    neg_d = nc.dram_tensor("negbig", [1, O], f32, kind="ExternalInput")
    idx_d = nc.dram_tensor("idx", [CHUNK, totch * 8], i16, kind="ExternalInput")
    dloc_d = nc.dram_tensor("dstloc", [CHUNK, totch], f32, kind="ExternalInput")
    ddv_d = nc.dram_tensor("dinvdst", [CHUNK, totch], f32, kind="ExternalInput")
    dvc_d = nc.dram_tensor("dinvcols", [CHUNK, NPC // CHUNK], f32,
                           kind="ExternalInput")
    dmask_d = nc.dram_tensor("dmask", [1, NPC], f32, kind="ExternalInput")
    dvrep_d = nc.dram_tensor("dvrep", [O, NPC], f32, kind="ExternalInput")
    out_d = nc.dram_tensor("out", [O, 1], f32, kind="ExternalOutput")

    dt_mem = bf16 if use_bf16 else f32
    gsl = [nc.dram_tensor(f"gsl{i}", [NPC, H], dt_mem, kind="Internal")
           for i in range(3)]
    gfull = [nc.dram_tensor(f"gfull{i}", [NPAD, H], dt_mem, kind="Internal",
                            addr_space="Shared") for i in range(3)]

    with tile.TileContext(nc) as tc:
        from concourse.library_config import mlp as _mlp_lib
        nc.gpsimd.load_library(_mlp_lib)
        with (
            tc.tile_pool(name="const", bufs=1) as cpool,
            tc.tile_pool(name="psagg", bufs=2, space="PSUM") as psagg,
            tc.tile_pool(name="pstr", bufs=2, space="PSUM") as pstr,
            tc.tile_pool(name="psfin", bufs=2, space="PSUM") as psfin,
        ):
            # resident constants
            w0_sb = [cpool.tile([CHUNK, H], f32, name=f"w0_{k}")
                     for k in range(KH)]
            for k in range(KH):
                nc.sync.dma_start(w0_sb[k][:, :],
                                  w0_d.ap()[k * CHUNK:(k + 1) * CHUNK, :])
            w1_sb = cpool.tile([H, H], f32, name="w1")
            nc.sync.dma_start(w1_sb[:, :], w1_d.ap()[:, :])
            w2_sb = cpool.tile([H, H], f32, name="w2")
            nc.sync.dma_start(w2_sb[:, :], w2_d.ap()[:, :])
            wl_sb = cpool.tile([H, O], f32, name="wl")
            nc.sync.dma_start(wl_sb[:, :], wl_d.ap()[:, :])
            b_sb = []
            for i in range(3):
                bt = cpool.tile([H, 1], f32, name=f"b{i}")
                nc.sync.dma_start(bt[:, :], b_d[i].ap()[:, :])
                b_sb.append(bt)
            iota_sb = cpool.tile([CHUNK, TILE_D], f32, name="iota")
            nc.sync.dma_start(iota_sb[:, :], iota_d.ap()[:, :])
            neg_sb = cpool.tile([1, O], f32, name="negbig")
            nc.sync.dma_start(neg_sb[:, :], neg_d.ap()[:, :])
            idx_sb = cpool.tile([CHUNK, totch * 8], i16, name="idx")
            nc.sync.dma_start(idx_sb[:, :], idx_d.ap()[:, :])
            dloc_sb = cpool.tile([CHUNK, totch], f32, name="dstloc")
            nc.sync.dma_start(dloc_sb[:, :], dloc_d.ap()[:, :])
            ddv_sb = cpool.tile([CHUNK, totch], f32, name="dinvdst")
            nc.sync.dma_start(ddv_sb[:, :], ddv_d.ap()[:, :])
            dvc_sb = cpool.tile([CHUNK, NPC // CHUNK], f32, name="dinvcols")
            nc.sync.dma_start(dvc_sb[:, :], dvc_d.ap()[:, :])
            dmask_sb = cpool.tile([1, NPC], f32, name="dmask")
            nc.sync.dma_start(dmask_sb[:, :], dmask_d.ap()[:, :])
            dvrep_sb = cpool.tile([O, NPC], f32, name="dvrep")
            nc.sync.dma_start(dvrep_sb[:, :], dvrep_d.ap()[:, :])
            runmax = cpool.tile([O, T], f32, name="runmax")
            constP = None
            if dbg_nop:
                constP = cpool.tile([CHUNK, TILE_D], dt_g, name="constP")
                nc.vector.tensor_scalar(
                    constP[:, :], iota_sb[:, :], dloc_sb[:, 0:1],
                    ddv_sb[:, 0:1], mybir.AluOpType.is_equal,
                    mybir.AluOpType.mult)
            if dbg_onlygather:
                nc.vector.memset(runmax[:, :], 0.0)

            # ---- init transform (once): g0 = (dinv*x) @ W0, node-major.
            # xt lives in its own pool so the space is recycled for the
            # per-layer pools below.
            with (tc.tile_pool(name="xtp", bufs=1) as xtpool,
                  tc.tile_pool(name="goutp", bufs=4) as opool0):
                xt_sb = [xtpool.tile([CHUNK, NPC], f32, name=f"xt{k}")
                         for k in range(KH)]
                for k in range(KH):
                    nc.sync.dma_start(xt_sb[k][:, :],
                                      xt_d.ap()[k * CHUNK:(k + 1) * CHUNK, :])
                for j in range(NPC // CHUNK):
                    ps = pstr.tile([CHUNK, H], f32)
                    for k in range(KH):
                        nc.tensor.matmul(
                            ps[:, :],
                            lhsT=xt_sb[k][:, j * CHUNK:(j + 1) * CHUNK],
                            rhs=w0_sb[k][:, :],
                            start=(k == 0), stop=(k == KH - 1))
                    gc = opool0.tile([CHUNK, H], dt_mem)
                    nc.scalar.activation(gc[:, :], ps[:, :],
                                         mybir.ActivationFunctionType.Copy)
                    nc.sync.dma_start(
                        gsl[0].ap()[j * CHUNK:(j + 1) * CHUNK, :], gc[:, :])
            if dbg_nocoll:
                for cc in range(NCORES):
                    nc.sync.dma_start(
                        gfull[0].ap()[cc * NPC:(cc + 1) * NPC, :],
                        gsl[0].ap()[:, :])
            else:
                nc.gpsimd.collective_compute(
                    "AllGather", mybir.AluOpType.bypass,
                    replica_groups=[list(range(NCORES))],
                    ins=[gsl[0].ap()], outs=[gfull[0].ap()])

            with (
                tc.tile_pool(name="gather", bufs=(5 if use_bf16 else 4)) as gpool,
                tc.tile_pool(name="pmat", bufs=3) as ppool,
                tc.tile_pool(name="stile", bufs=3) as spool,
                tc.tile_pool(name="gout", bufs=4) as opool,
                tc.tile_pool(name="fin", bufs=2) as fpool,
            ):
                W_next = {0: w1_sb, 1: w2_sb}
                for _rep in range(reps):
                    pbuild = 0
                    for l in range(3):
                        cbase = 0
                        idxoff = 0
                        gq = 0
                        for t in range(T):
                            nch_t = [n_lo[t], n_hi[t]]
                            gt = []
                            pms = []
                            cb_hf = [cbase, cbase + nch_t[0]]
                            for hf in range(2):
                                nch = nch_t[hf]
                                gtile = gpool.tile([CHUNK, nch * H], dt_g,
                                                   tag=f"g{hf}")
                                nid = nch * CHUNK
                                if dbg_nogather:
                                    nc.sync.dma_start(
                                        gtile[:, :].rearrange(
                                            "p (c e) -> p c e", e=H),
                                        gfull[l].ap()[:nid, :].bitcast(dt_g)
                                        .rearrange("(c p) e -> p c e", p=CHUNK))
                                else:
                                    nc.gpsimd.dma_gather(
                                        out_ap=gtile[:, :].rearrange(
                                            "p (c e) -> p c e", e=H),
                                        in_ap=gfull[l].ap()
                                        [hf * HALF:(hf + 1) * HALF, :]
                                        .bitcast(dt_g),
                                        idxs_ap=idx_sb[:, idxoff:
                                                       idxoff + nid // 16],
                                        num_idxs=nid, num_idxs_reg=nid,
                                        elem_size=H, single_packet=False,
                                        queue_num=gq % n_queues)
                                gq += 1
                                gt.append(gtile)
                                idxoff += nid // 16
                                use_multi = (l == 2) or not has_bias01
                                if use_multi and not (dbg_onlygather or
                                                      dbg_noagg or dbg_nop):
                                    Pm = ppool.tile(
                                        [CHUNK, nch * TILE_D], dt_g, tag="pm")
                                    pv = Pm[:, :].rearrange(
                                        "p (g d) -> p g d", d=TILE_D)
                                    dl = dloc_sb[:, cb_hf[hf]:cb_hf[hf] + nch]\
                                        .broadcast_to([CHUNK, nch, TILE_D])
                                    ioap = iota_sb[:, :]
                                    io = ioap.__replace__(
                                        ap=[list(ioap.ap[0]), [0, nch],
                                            list(ioap.ap[1])])
                                    pbuild += 1
                                    nc.vector.scalar_tensor_tensor(
                                        pv, dl, 0.0, io,
                                        mybir.AluOpType.add,
                                        mybir.AluOpType.is_equal)
                                    if l == 2 and not b2zero:
                                        dv = ddv_sb[:, cb_hf[hf]:
                                                    cb_hf[hf] + nch]\
                                            .broadcast_to(
                                                [CHUNK, nch, TILE_D])
                                        nc.vector.tensor_tensor(
                                            out=pv, in0=pv, in1=dv,
                                            op=mybir.AluOpType.mult)
                                    pms.append(Pm)
                                else:
                                    pms.append(None)
                            ps = psagg.tile([H, TILE_D], f32)
                            ntot = nch_t[0] + nch_t[1]
                            if dbg_onlygather:
                                cbase += ntot
                                continue
                            if dbg_noagg:
                                # skip P-builds and chunk matmuls; fake psum
                                nc.tensor.matmul(
                                    ps[:, :], lhsT=gt[0][:, 0:H],
                                    rhs=gt[0][:, 0:TILE_D],
                                    start=True, stop=True)
                            for k in range([0, ntot][not dbg_noagg]):
                                hf = 0 if k < nch_t[0] else 1
                                kk = k if hf == 0 else k - nch_t[0]
                                cg = cbase + k
                                if dbg_nop:
                                    P = constP
                                elif pms[hf] is not None:
                                    P = pms[hf][:, kk * TILE_D:
                                                (kk + 1) * TILE_D]
                                else:
                                    P = ppool.tile([CHUNK, TILE_D], dt_g)
                                    nc.vector.tensor_scalar(
                                        P[:, :], iota_sb[:, :],
                                        dloc_sb[:, cg:cg + 1],
                                        ddv_sb[:, cg:cg + 1],
                                        mybir.AluOpType.is_equal,
                                        mybir.AluOpType.mult)
                                nc.tensor.matmul(
                                    ps[:, :],
                                    lhsT=gt[hf][:, kk * H:(kk + 1) * H],
                                    rhs=P if pms[hf] is not None and not dbg_nop
                                    else P[:, :],
                                    start=(k == 0), stop=(k == ntot - 1))
                            cbase += ntot
                            # evict: h^T = psum + b_l (bias per feat partition)
                            sT = spool.tile([H, TILE_D], f32)
                            if l == 2 or has_bias01:
                                nc.scalar.activation(
                                    sT[:, :], ps[:, :],
                                    mybir.ActivationFunctionType.Identity,
                                    bias=b_sb[l][:, 0:1])
                            else:
                                nc.scalar.activation(
                                    sT[:, :], ps[:, :],
                                    mybir.ActivationFunctionType.Copy)
                            if l < 2:
                                for j2 in range(TILE_D // CHUNK):
                                    jn = t * (TILE_D // CHUNK) + j2
                                    ps2 = pstr.tile([CHUNK, H], f32)
                                    nc.tensor.matmul(
                                        ps2[:, :],
                                        lhsT=sT[:, j2 * CHUNK:(j2 + 1) * CHUNK],
                                        rhs=W_next[l][:, :],
                                        start=True, stop=True)
                                    gc = opool.tile([CHUNK, H], dt_mem)
                                    nc.scalar.activation(
                                        gc[:, :], ps2[:, :],
                                        mybir.ActivationFunctionType.Copy,
                                        scale=dvc_sb[:, jn:jn + 1])
                                    nc.sync.dma_start(
                                        gsl[l + 1].ap()
                                        [jn * CHUNK:(jn + 1) * CHUNK, :],
                                        gc[:, :])
                            else:
                                psf = psfin.tile([O, TILE_D], f32)
                                nc.tensor.matmul(psf[:, :], lhsT=wl_sb[:, :],
                                                 rhs=sT[:, :], start=True,
                                                 stop=dbg_nomask)
                                if not dbg_nomask:
                                    nc.tensor.matmul(
                                        psf[:, :], lhsT=neg_sb[:, :],
                                        rhs=dmask_sb[0:1,
                                                     t * TILE_D:(t + 1) * TILE_D],
                                        start=False, stop=True)
                                if b2zero:
                                    # dinv[dst] applied per column here (P is
                                    # a plain one-hot); dummies have dvrep=1
                                    # so the NEG_BIG mask survives the scale.
                                    psf2 = fpool.tile([O, TILE_D], f32)
                                    nc.vector.tensor_tensor(
                                        out=psf2[:, :], in0=psf[:, :],
                                        in1=dvrep_sb[:, t * TILE_D:
                                                     (t + 1) * TILE_D],
                                        op=mybir.AluOpType.mult)
                                    nc.vector.reduce_max(
                                        out=runmax[:, t:t + 1],
                                        in_=psf2[:, :],
                                        axis=mybir.AxisListType.X)
                                else:
                                    nc.vector.reduce_max(
                                        out=runmax[:, t:t + 1], in_=psf[:, :],
                                        axis=mybir.AxisListType.X)
                        if l < 2:
                            if dbg_onlygather:
                                continue
                            if dbg_nocoll:
                                for cc in range(NCORES):
                                    nc.sync.dma_start(
                                        gfull[l + 1].ap()
                                        [cc * NPC:(cc + 1) * NPC, :],
                                        gsl[l + 1].ap()[:, :])
                            else:
                                nc.gpsimd.collective_compute(
                                    "AllGather", mybir.AluOpType.bypass,
                                    replica_groups=[list(range(NCORES))],
                                    ins=[gsl[l + 1].ap()],
                                    outs=[gfull[l + 1].ap()])

                    fin = cpool.tile([O, 1], f32, name=f"fin{_rep}")
                    nc.vector.reduce_max(out=fin[:, :], in_=runmax[:, :],
                                         axis=mybir.AxisListType.X)
                    nc.sync.dma_start(out_d.ap()[:, :], fin[:, :])

    nc.compile()
    return nc


# ---------------------------------------------------------------- entry

def kernel(x, edge_index, batch, W0, b0, W1, b1, W2, b2, Wlin, blin):
    from concourse.bass_utils import run_bass_kernel_spmd

    meta, in_maps = _preprocess(np.asarray(x), np.asarray(edge_index),
                                W0, b0, W1, b1, W2, b2, Wlin, blin)
    nc = _compiled_cache.get(meta)
    if nc is None:
        nc = _build(meta)
        _compiled_cache[meta] = nc
    res = run_bass_kernel_spmd(nc, in_maps, core_ids=list(range(NCORES)))
    parts = np.stack([r["out"][:, 0] for r in res.results])  # [cores, O]
    out = parts.max(axis=0) + np.asarray(blin, np.float32)
    return out.reshape(1, O).astype(np.float32)

